# revision 2
# baseline (speedup 1.0000x reference)
"""Trainium2 Bass kernel for a dense transformer encoder layer.

Reference computation (fp32):
    q,k,v = x@Wq+bq, x@Wk+bk, x@Wv+bv           (16 heads, dk=64)
    att   = softmax(q k^T / 8) v ; att_out = att@Wo + bo
    x2    = LN(x + att_out; g1, be1)
    out   = LN(x2 + relu(x2@W1 + b1)@W2 + b2; g2, be2)

Sharding: pure data parallel over the 8 cores. Core i handles batch b=i//2,
query half h=i%2 (1024 query tokens), with the full 2048-token K/V context of
its batch element. No collectives.

On-chip layout is feature-major ("transposed"): activations live as
[d, tokens] so every matmul contraction lands on the partition dim and no
on-device transposes are needed anywhere.

Precision strategy (rel-err budget 2e-2; bf16 baseline measures 1.7e-3):
the attention branch contributes only ~1.3% of the residual-stream scale
(weights are 0.02-scale), so the whole attention pipeline runs fp8e4m3:
QKV + out projections use fp8 DoubleRow matmuls (2 k-chunks per MM), and
att@V uses DoubleRow over k-tile pairs with exp-scores E stored as fp8.
The FFN branch is ~50% of the stream, so FFN1/FFN2 stay bf16.

Attention restructure vs the bf16 baseline:
- Head PAIRS (h, h^1) share each scores PSUM tile [128, 1024] = [h0|h1];
  the two K=64 score matmuls land on PE row-groups 0-63 / 64-127
  (tile_position auto-derived from partition offsets) and run CONCURRENTLY
  in the array, recovering the 2x an M=64 contraction otherwise wastes.
- The softmax exp (33.5M elems/core, the old ACT bottleneck at ~285us) is
  split across ACT (table exp -> fp8) and DVE (Schraudolph: fp8 bits =
  int8(score * 1.4427 + 55.55), one fused mult+add tensor_scalar). Per-kp
  tiles are engine-homogeneous so attV matmuls wait on <= 2 semaphores.
- Softmax skips max-subtraction (scores ~ N(0, 0.41); exp can't overflow)
  and normalization rides the attV matmul as a ones-column in V (row 64 of
  the DoubleRow output is the denominator).
"""

import os
import sys

if "/opt/trn_rl_repo" not in sys.path:
    sys.path.insert(0, "/opt/trn_rl_repo")

import numpy as np
import ml_dtypes

P = 128
D = 1024            # d_model
DC = D // P         # 8 chunks of d_model
H = 16              # heads
DK = 64             # head dim
F = 4096            # d_ff
FC = F // P         # 32 chunks of d_ff
FG = 4              # d_ff streaming groups (of 1024)
SQ = 1024           # query tokens per core
SKV = 2048          # key/value tokens per core
KT = SKV // P       # 16 key-token tiles
VROW = H * (DK + 1) + 64   # vP row bytes per k-tile (pad to %16 for DoubleRow)
EPS = 1e-5
N_CORES = 8
B, S = 4, 2048

BF16 = ml_dtypes.bfloat16
F8 = ml_dtypes.float8_e4m3

# Schraudolph exp -> fp8e4m3 bits: i8 = round(score * SCH_A + SCH_B)
# (score pre-1/8-scale; e4m3 bias 7: i8 = 8*(score/8/ln2 + 7) + c, c=-0.45)
SCH_A = 1.442695
SCH_B = 55.55
# kps whose two exp tiles run on ACT (rest on DVE); engine-homogeneous per
# kp so each E tile has a single writer engine (2-sem-wait limit on attV).
ACT_KPS = (1, 4, 6)

_CACHE = {}


def build_nc(phases=4):
    """Build the single-core Bass/Tile program (SPMD: same program, per-core
    data)."""
    import concourse.bass as bass
    import concourse.mybir as mybir
    import concourse.tile as tile
    from concourse import bacc

    f32 = mybir.dt.float32
    bf = mybir.dt.bfloat16
    f16 = mybir.dt.float16
    f8 = mybir.dt.float8e4
    i8 = mybir.dt.int8
    AF = mybir.ActivationFunctionType
    ALU = mybir.AluOpType
    DR = mybir.MatmulPerfMode.DoubleRow

    nc = bacc.Bacc("TRN2", target_bir_lowering=False, debug=False)

    def din(name, shape, dt):
        return nc.dram_tensor(name, shape, dt, kind="ExternalInput").ap()

    xqT = din("xqT", [D, SQ], f32)       # x query-shard, transposed [d, sq]
    xq8 = din("xq8", [D, SQ], f8)        # same, fp8
    xkv8 = din("xkv8", [D, SKV], f8)     # full-context x, transposed, fp8
    wq = din("wq", [D, D], f8)
    wk = din("wk", [D, D], f8)
    wv = din("wv", [D, D], f8)
    wo = din("wo", [D, D], f8)
    w1 = din("w1", [D, F], bf)
    w2 = din("w2", [F, D], bf)
    cvec = din("cvec", [P, 97], f32)     # host-packed per-partition consts
    bvb = din("bvb", [P, D], f32)        # bv broadcast to 128 partitions (host)
    out = nc.dram_tensor("out", [D, SQ], f32, kind="ExternalOutput").ap()

    def pc(ap1d):  # [D] -> [P, DC] partition/chunk layout
        return ap1d.rearrange("(c p) -> p c", p=P)

    def pcs(ap2d, n):  # [D, n] -> [P, DC, n]
        return ap2d.rearrange("(c p) s -> p c s", p=P)

    with tile.TileContext(nc) as tc:
        # ------------- pools: two-sided LIFO schedule for SBUF reuse ---------
        constp_cm = tc.tile_pool(name="constp", bufs=1)
        constp = constp_cm.__enter__()
        pw_cm = tc.tile_pool(name="pw", bufs=2)
        pw = pw_cm.__enter__()
        pshare_cm = tc.tile_pool(name="pshare", bufs=1)
        pshare = pshare_cm.__enter__()
        pqkv_cm = tc.tile_pool(name="pqkv", bufs=1)
        pqkv = pqkv_cm.__enter__()
        pin_cm = tc.tile_pool(name="pin", bufs=1)
        pin = pin_cm.__enter__()
        ppB_cm = tc.tile_pool(name="ppB", bufs=4, space="PSUM")
        ppB = ppB_cm.__enter__()

        # packed small constants: ONE dma (single writer -> single wait for
        # readers; walrus allows at most 2 sem waits per instruction)
        cpk = constp.tile([P, 128], f32, tag="cpk", name="cpk")
        nc.sync.dma_start(cpk[:, 0:97], cvec)
        bqT = cpk[:, 0:8]
        bkT = cpk[:, 8:16]
        boT = cpk[:, 16:24]
        b2T = cpk[:, 24:32]
        g1T = cpk[:, 32:40]
        be1T = cpk[:, 40:48]
        g2T = cpk[:, 48:56]
        be2T = cpk[:, 56:64]
        b1T = cpk[:, 64:96]
        eps_col = cpk[:, 96:97]
        cpr = constp.tile([P, 392], f16, tag="cpr", name="cpr")
        nc.vector.memset(cpr, 0.0)
        nc.vector.memset(cpr[:, 0:1], 1.0)            # ones column [P,1]
        nc.vector.memset(cpr[0:1, 136:264], 1.0 / D)  # e_mu row 0
        nc.vector.memset(cpr[32:33, 264:392], 1.0 / D)  # e_ss row 32
        ones_col = cpr[:, 0:1]
        e_mu = cpr[0:33, 136:264]         # [33, 128]: row0=1/D, rest 0
        e_ss = cpr[0:33, 264:392]         # [33, 128]: row32=1/D, rest 0
        bvb_t = constp.tile([P, D], f32, tag="bvb", name="bvb_t")
        nc.sync.dma_start(bvb_t, bvb)
        # absorber reads: advance ACT's vector clock past the const writers so
        # later bias reads don't stack a third wait on top of PE+DMA.
        scr = constp.tile([1, 4], f32, tag="scr", name="scr")
        nc.scalar.activation(scr[0:1, 0:1], cpk[0:1, 0:1], AF.Copy)
        nc.scalar.activation(scr[0:1, 1:2], bvb_t[0:1, 0:1], AF.Copy)
        nc.scalar.activation(scr[0:1, 2:3], cpr[0:1, 0:1], AF.Copy)

        # ======================= Phase B: QKV projections =====================
        def pe_absorb(t2d):
            nc.tensor.ldweights(t2d[0:1, 0:1])

        xq8_t = pshare.tile([P, DC, SQ], f8, tag="share8", name="xq8_t")
        xkv8_t = pin.tile([P, DC, SKV], f8, tag="xkv8", name="xkv8_t")
        nc.sync.dma_start(xq8_t, pcs(xq8, SQ))
        nc.sync.dma_start(xkv8_t, pcs(xkv8, SKV))
        pe_absorb(xq8_t[:, 0, :])
        pe_absorb(xkv8_t[:, 0, :])

        qT = pqkv.tile([P, DC, SQ], bf, tag="qT", name="qT")
        kTt = pqkv.tile([P, DC, SKV], bf, tag="kTt", name="kTt")
        # v packed per head: 64 v-dims + ones column at slot 64 (softmax
        # denominator rides the attV matmul); row padded to VROW (%16) so
        # DoubleRow's k-tile-pair stride is legal, and so the [128,128]
        # stationary slice for the last head stays in bounds.
        vP = pqkv.tile([P, KT, VROW], f8, tag="vP", name="vP")
        vPh = vP[:, :, :H * (DK + 1)].rearrange(
            "p t (h e) -> p t h e", e=DK + 1)  # view, 16 heads
        nc.vector.memset(vP[:, :, H * (DK + 1):], 0.0)
        nc.vector.memset(vPh[:, :, :, DK:DK + 1], 1.0)

        # v token-major [skv, dv] (+bias via broadcast tile), fp8 DoubleRow
        # over k-chunk pairs (4 MMs per psum group).
        wv_t = pw.tile([P, DC, D], f8, tag="w", name="wv_t")
        nc.sync.dma_start(wv_t, pcs(wv, D))
        pe_absorb(wv_t[:, 0, :])
        for t in range(KT):
            for j in range(D // 512):
                ps = ppB.tile([P, 512], f32, tag="psB", name="psB")
                for kc in range(DC // 2):
                    nc.tensor.matmul(
                        ps, lhsT=xkv8_t[:, 2 * kc:2 * kc + 2, t * P:(t + 1) * P],
                        rhs=wv_t[:, 2 * kc:2 * kc + 2, j * 512:(j + 1) * 512],
                        start=(kc == 0), stop=(kc == DC // 2 - 1),
                        perf_mode=DR)
                nc.vector.tensor_tensor(
                    vPh[:, t, j * 8:(j + 1) * 8, 0:DK],
                    ps.rearrange("p (h e) -> p h e", e=DK),
                    bvb_t[:, j * 512:(j + 1) * 512].rearrange(
                        "p (h e) -> p h e", e=DK),
                    ALU.add)
        # k^T [dk, skv]: all 8 chunks here (no attention-phase deferral; the
        # restructured attention keeps PE busy on its own).
        wk_t = pw.tile([P, DC, D], f8, tag="w", name="wk_t")
        nc.sync.dma_start(wk_t, pcs(wk, D))
        pe_absorb(wk_t[:, 0, :])
        for m in range(DC):
            for j in range(SKV // 512):
                ps = ppB.tile([P, 512], f32, tag="psB", name="psB")
                for kc in range(DC // 2):
                    nc.tensor.matmul(
                        ps, lhsT=wk_t[:, 2 * kc:2 * kc + 2, m * P:(m + 1) * P],
                        rhs=xkv8_t[:, 2 * kc:2 * kc + 2, j * 512:(j + 1) * 512],
                        start=(kc == 0), stop=(kc == DC // 2 - 1),
                        perf_mode=DR)
                nc.scalar.activation(kTt[:, m, j * 512:(j + 1) * 512], ps,
                                     AF.Identity, bias=bkT[:, m:m + 1])
        # q^T [dq, sq]
        wq_t = pw.tile([P, DC, D], f8, tag="w", name="wq_t")
        nc.sync.dma_start(wq_t, pcs(wq, D))
        pe_absorb(wq_t[:, 0, :])
        for m in range(DC):
            for j in range(SQ // 512):
                ps = ppB.tile([P, 512], f32, tag="psB", name="psB")
                for kc in range(DC // 2):
                    nc.tensor.matmul(
                        ps, lhsT=wq_t[:, 2 * kc:2 * kc + 2, m * P:(m + 1) * P],
                        rhs=xq8_t[:, 2 * kc:2 * kc + 2, j * 512:(j + 1) * 512],
                        start=(kc == 0), stop=(kc == DC // 2 - 1),
                        perf_mode=DR)
                nc.scalar.activation(qT[:, m, j * 512:(j + 1) * 512], ps,
                                     AF.Identity, bias=bqT[:, m:m + 1])

        ppB_cm.__exit__(None, None, None)

        outr = pcs(out, SQ)
        if phases <= 1:
            for c in range(DC):
                nc.sync.dma_start(outr[:, c, :], qT.bitcast(f32)[:, c, :512])
            for cm in (pin_cm, pqkv_cm, pshare_cm, pw_cm, constp_cm):
                cm.__exit__(None, None, None)
            nc.compile()
            return nc

        # ===================== Phase C: attention, head pairs =================
        # Per pair-group (p, qh): 16 score tiles ps(kt) [128k, h0q|h1q] (the 2
        # K=64 MMs overlap on PE row-groups 0/64), exp'd into per-kp E tiles
        # [128, 2048] = [kt0:h0|h1, kt1:h0|h1] (fp8), then per head 8 attV
        # DoubleRow MMs contract k-tile PAIRS into up [128, 512] (rows 0:64 =
        # U', row 64 = denominator). Software pipeline: scores/exp(g) || attV
        # (g-1) || normalize(g-2).
        attT = pshare.tile([P, DC, SQ], f8, tag="share8", name="attT")

        pE_cm = tc.tile_pool(name="pE", bufs=18)
        pE = pE_cm.__enter__()
        tmpC_cm = tc.tile_pool(name="tmpC", bufs=2)
        tmpC = tmpC_cm.__enter__()
        ppC_s_cm = tc.tile_pool(name="ppC_s", bufs=3, space="PSUM")
        ppC_s = ppC_s_cm.__enter__()
        ppC_u_cm = tc.tile_pool(name="ppC_u", bufs=2, space="PSUM")
        ppC_u = ppC_u_cm.__enter__()

        def emit_normalize(pend):
            """Scale U' rows 0:64 by 1/colsum (row 64) and write into attT
            (fp8, feeding the DoubleRow out-projection)."""
            h, qh, up = pend
            c_h, off = h // 2, (h % 2) * DK
            qs = qh * 512
            u_sb = tmpC.tile([DK + 1, 512], f32, tag="u_sb", name="u_sb")
            nc.vector.tensor_copy(u_sb, up[0:DK + 1, :])
            with nc.allow_low_precision(reason="1/colsum, |err| ~1e-6 of att"):
                nc.vector.reciprocal(u_sb[DK:DK + 1, :], u_sb[DK:DK + 1, :])
            # gpsimd broadcast only honors a partition-0 source; DMA the row
            # down from partition 64 (tiny sbuf->sbuf copy, off critical path)
            cs0 = tmpC.tile([1, 512], f32, tag="cs0", name="cs0")
            nc.sync.dma_start(cs0, u_sb[DK:DK + 1, :])
            nb_sb = tmpC.tile([DK, 512], f32, tag="nb_sb", name="nb_sb")
            nc.gpsimd.partition_broadcast(nb_sb, cs0)
            if off == 0:
                nc.vector.tensor_tensor(attT[0:DK, c_h, qs:qs + 512],
                                        u_sb[0:DK, :], nb_sb, ALU.mult)
            else:
                # engines cannot shift partitions; bounce through DMA
                atmp = tmpC.tile([DK, 512], f8, tag="atmp", name="atmp",
                                 bufs=1)
                nc.vector.tensor_tensor(atmp, u_sb[0:DK, :], nb_sb, ALU.mult)
                nc.sync.dma_start(attT[DK:P, c_h, qs:qs + 512], atmp)

        from concourse.tile_rust import add_dep_helper

        groups = [(p, qh) for p in range(DC) for qh in range(SQ // 512)]
        prev_block_last = None
        pend_av = None      # (p, qh, E4s)
        pend_norm = []      # [(h, qh, up)]

        def chain(mm):
            nonlocal prev_block_last
            if prev_block_last is not None:
                add_dep_helper(mm.ins, prev_block_last, sync=False,
                               reason="attention block order")
            prev_block_last = mm.ins

        for gi, (p, qh) in enumerate(groups):
            qs = qh * 512
            E4s = []
            if pend_av is not None:
                pp, pqh, pE4s = pend_av
                ups = [ppC_u.tile([P, 512], f32, tag="up", name="up", bufs=2)
                       for _ in range(2)]
            for kt in range(KT):
                # ---- scores for the head pair: one [128,1024] psum tile ----
                ps = ppC_s.tile([P, 1024], f32, tag="sc", name="sc")
                for h01 in range(2):
                    off = h01 * DK
                    mm = nc.tensor.matmul(
                        ps[:, h01 * 512:(h01 + 1) * 512],
                        lhsT=kTt[off:off + DK, p, kt * P:(kt + 1) * P],
                        rhs=qT[off:off + DK, p, qs:qs + 512],
                        start=True, stop=True)
                    chain(mm)
                # ---- exp into the kp's E tile (engine per ACT_KPS) ----
                kp, half = kt // 2, kt % 2
                if half == 0:
                    E4 = pE.tile([P, 2048], f8, tag="E4", name="E4")
                    E4s.append(E4)
                dst = E4s[kp][:, half * 1024:(half + 1) * 1024]
                if kp in ACT_KPS:
                    nc.scalar.activation(dst, ps, AF.Exp, scale=0.125)
                else:
                    with nc.allow_low_precision(
                            reason="Schraudolph fp8 exp; ~3% per softmax "
                                   "weight, attention branch is 1.3% of "
                                   "the residual stream"):
                        nc.vector.tensor_scalar(
                            dst.bitcast(i8), ps, SCH_A, SCH_B,
                            ALU.mult, ALU.add)
                # ---- interleave one attV DoubleRow MM of the previous group
                if pend_av is not None:
                    ah, akp = kt // 8, kt % 8
                    h = 2 * pp + ah
                    av = nc.tensor.matmul(
                        ups[ah],
                        lhsT=vP[:, 2 * akp:2 * akp + 2,
                                h * (DK + 1):h * (DK + 1) + P],
                        rhs=pE4s[akp].rearrange(
                            "p (t h q) -> p t h q", t=2, q=512)[:, :, ah, :],
                        start=(akp == 0), stop=(akp == 7),
                        perf_mode=DR)
                    chain(av)
                    if akp == 7:
                        pend_norm.append((h, pqh, ups[ah]))
            pend_av = (p, qh, E4s)
            while len(pend_norm) > 2:
                emit_normalize(pend_norm.pop(0))
        # drain: attV for the last group
        pp, pqh, pE4s = pend_av
        ups = [ppC_u.tile([P, 512], f32, tag="up", name="up", bufs=2)
               for _ in range(2)]
        for ah in range(2):
            h = 2 * pp + ah
            for akp in range(8):
                av = nc.tensor.matmul(
                    ups[ah],
                    lhsT=vP[:, 2 * akp:2 * akp + 2,
                            h * (DK + 1):h * (DK + 1) + P],
                    rhs=pE4s[akp].rearrange(
                        "p (t h q) -> p t h q", t=2, q=512)[:, :, ah, :],
                    start=(akp == 0), stop=(akp == 7),
                    perf_mode=DR)
                chain(av)
            pend_norm.append((h, pqh, ups[ah]))
        for pend in pend_norm:
            emit_normalize(pend)

        ppC_u_cm.__exit__(None, None, None)
        ppC_s_cm.__exit__(None, None, None)
        tmpC_cm.__exit__(None, None, None)
        pE_cm.__exit__(None, None, None)
        pin_cm.__exit__(None, None, None)
        pqkv_cm.__exit__(None, None, None)
        if phases <= 2:
            for c in range(DC):
                nc.sync.dma_start(outr[:, c, :],
                                  attT.bitcast(f32)[:, c, :256])
            for cm in (pshare_cm, pw_cm, constp_cm):
                cm.__exit__(None, None, None)
            nc.compile()
            return nc

        # =================== Phase D: out-proj + residual + LN1 ===============
        pxD_cm = tc.tile_pool(name="pxD", bufs=1)
        pxD = pxD_cm.__enter__()
        xq_f = pxD.tile([P, DC, SQ], f32, tag="xq_f", name="xq_f")
        nc.sync.dma_start(xq_f, pcs(xqT, SQ))
        py1_cm = tc.tile_pool(name="py1", bufs=1, side="right")
        py1 = py1_cm.__enter__()
        ppD_cm = tc.tile_pool(name="ppD", bufs=4, space="PSUM")
        ppD = ppD_cm.__enter__()

        y1 = py1.tile([P, DC, SQ], f32, tag="y1x2", name="y1")

        # xb = x + bo (in place on xq_f; per-partition bias)
        for c in range(DC):
            nc.scalar.activation(xq_f[:, c, :], xq_f[:, c, :], AF.Identity,
                                 bias=boT[:, c:c + 1])
        wo_t = pw.tile([P, DC, D], f8, tag="w", name="wo_t")
        nc.sync.dma_start(wo_t, pcs(wo, D))
        pe_absorb(wo_t[:, 0, :])
        for j in range(SQ // 512):
            for m in range(DC):
                ps = ppD.tile([P, 512], f32, tag="psD", name="psD")
                for kc in range(DC // 2):
                    nc.tensor.matmul(
                        ps, lhsT=wo_t[:, 2 * kc:2 * kc + 2, m * P:(m + 1) * P],
                        rhs=attT[:, 2 * kc:2 * kc + 2, j * 512:(j + 1) * 512],
                        start=(kc == 0), stop=(kc == DC // 2 - 1),
                        perf_mode=DR)
                nc.vector.tensor_add(y1[:, m, j * 512:(j + 1) * 512], ps,
                                     xq_f[:, m, j * 512:(j + 1) * 512])
        ppD_cm.__exit__(None, None, None)
        pxD_cm.__exit__(None, None, None)
        pshare_cm.__exit__(None, None, None)
        pw_cm.__exit__(None, None, None)

        def layernorm_j(src, dst, gT, beT, j, post=None):
            """Feature-dim layernorm for column half j (src/dst may alias).

            post(c, sl) runs after each chunk of dst is written (e.g. bf16
            cast or output DMA) so downstream work starts per-chunk.
            """
            sl = slice(j * 512, (j + 1) * 512)
            tmp_cm = tc.tile_pool(name="tmpLN", bufs=2, side="right")
            tmp = tmp_cm.__enter__()
            pps_cm = tc.tile_pool(name="pps", bufs=1, space="PSUM")
            pps = pps_cm.__enter__()
            stats = pps.tile([33, 512], f32, tag="stats", name="stats")
            for c in range(DC):
                yh = tmp.tile([P, 512], f16, tag="yh", name="yh", bufs=3)
                nc.vector.tensor_copy(yh, src[:, c, sl])
                nc.tensor.matmul(stats[0:1, :], lhsT=ones_col, rhs=yh,
                                 start=(c == 0), stop=(c == DC - 1))
                sq = tmp.tile([P, 512], f16, tag="sq", name="sq", bufs=3)
                nc.vector.tensor_mul(sq, yh, yh)
                nc.tensor.matmul(stats[32:33, :], lhsT=ones_col, rhs=sq,
                                 start=(c == 0), stop=(c == DC - 1))
            stats_sb = tmp.tile([33, 512], f16, tag="stats_sb",
                                name="stats_sb", bufs=1)
            nc.vector.memset(stats_sb, 0.0)
            nc.scalar.activation(stats_sb[0:1, :], stats[0:1, :], AF.Copy)
            nc.scalar.activation(stats_sb[32:33, :], stats[32:33, :], AF.Copy)
            pps_cm.__exit__(None, None, None)

            ppb_cm = tc.tile_pool(name="ppb", bufs=1, space="PSUM")
            ppb = ppb_cm.__enter__()
            mu_b = ppb.tile([P, 512], f32, tag="mu_b", name="mu_b")
            nc.tensor.matmul(mu_b, lhsT=e_mu, rhs=stats_sb,
                             start=True, stop=True)
            ms_b = ppb.tile([P, 512], f32, tag="ms_b", name="ms_b")
            nc.tensor.matmul(ms_b, lhsT=e_ss, rhs=stats_sb,
                             start=True, stop=True)
            # var = E[y^2] - mu^2 ; rstd = 1/sqrt(var+eps)
            mu_sb = tmp.tile([P, 512], f32, tag="mu_sb", name="mu_sb", bufs=1)
            nc.scalar.activation(mu_sb, mu_b, AF.Copy)
            t = tmp.tile([P, 512], f32, tag="t_var", name="t_var", bufs=1)
            nc.vector.tensor_mul(t, mu_sb, mu_sb)
            nc.vector.tensor_sub(t, ms_b, t)
            nc.scalar.activation(t, t, AF.Sqrt, bias=eps_col)
            rstd = tmp.tile([P, 512], f32, tag="rstd", name="rstd", bufs=1)
            nc.vector.reciprocal(rstd, t)
            ppb_cm.__exit__(None, None, None)
            for c in range(DC):
                t1 = tmp.tile([P, 512], f32, tag="t1", name="t1", bufs=3)
                nc.vector.tensor_sub(t1, src[:, c, sl], mu_sb)
                nc.vector.tensor_mul(t1, t1, rstd)
                nc.scalar.activation(dst[:, c, sl], t1, AF.Identity,
                                     bias=beT[:, c:c + 1],
                                     scale=gT[:, c:c + 1])
                if post is not None:
                    post(c, sl)
            tmp_cm.__exit__(None, None, None)

        # LN1 per column-half, fused bf16 cast, overlapping oproj's tail.
        px2b_cm = tc.tile_pool(name="px2b", bufs=1, side="right")
        px2b = px2b_cm.__enter__()
        x2b = px2b.tile([P, DC, SQ], bf, tag="x2b", name="x2b")

        def cast_post(c, sl):
            nc.vector.tensor_copy(x2b[:, c, sl], y1[:, c, sl])

        if phases <= 3:
            for j in range(SQ // 512):
                layernorm_j(y1, y1, g1T, be1T, j, post=cast_post)
            for c in range(DC):
                nc.sync.dma_start(outr[:, c, :], y1[:, c, :])
            px2b_cm.__exit__(None, None, None)
            py1_cm.__exit__(None, None, None)
            constp_cm.__exit__(None, None, None)
            nc.compile()
            return nc

        # ============================ Phase E: FFN ============================
        pwE_cm = tc.tile_pool(name="pwE", bufs=4, side="right")
        pwE = pwE_cm.__enter__()
        pffn_cm = tc.tile_pool(name="pffn", bufs=1, side="right")
        pffn = pffn_cm.__enter__()
        ppE_cm = tc.tile_pool(name="ppE", bufs=4, space="PSUM")
        ppE = ppE_cm.__enter__()

        hT = pffn.tile([P, FC, SQ], bf, tag="hT", name="hT")
        w1r = pcs(w1, F)
        w1_gs = []
        for g in range(FG):
            w1_g = pwE.tile([P, DC, 1024], bf, tag="wE", name="w1_g")
            nc.sync.dma_start(w1_g, w1r[:, :, g * 1024:(g + 1) * 1024])
            pe_absorb(w1_g[:, 0, :])
            w1_gs.append(w1_g)
        # LN1 half j feeds FFN1 half j immediately; the next LN half's
        # DVE/ACT chain then runs under FFN1's matmuls.
        for j in range(SQ // 512):
            layernorm_j(y1, y1, g1T, be1T, j, post=cast_post)
            for g in range(FG):
                for fl in range(8):
                    fm = g * 8 + fl
                    ps = ppE.tile([P, 512], f32, tag="psE", name="psE")
                    for kc in range(DC):
                        nc.tensor.matmul(
                            ps, lhsT=w1_gs[g][:, kc, fl * P:(fl + 1) * P],
                            rhs=x2b[:, kc, j * 512:(j + 1) * 512],
                            start=(kc == 0), stop=(kc == DC - 1))
                    nc.scalar.activation(hT[:, fm, j * 512:(j + 1) * 512], ps,
                                         AF.Relu, bias=b1T[:, fm:fm + 1])

        # x2 += b2 (residual carries the final bias; raw x2 no longer needed)
        for c in range(DC):
            nc.scalar.activation(y1[:, c, :], y1[:, c, :], AF.Identity,
                                 bias=b2T[:, c:c + 1])

        # FFN2: W2 fully resident (4 wE slots); per (j, m) one 32-matmul psum
        # accumulation over (g, kc), residual-added in place into y1 (=y2).
        # LN2 for half j runs right after its m-loop, overlapping j+1's FFN2.
        w2r = pcs(w2, D)
        w2_gs = []
        for g in range(FG):
            w2_g = pwE.tile([P, DC, 1024], bf, tag="wE", name="w2_g")
            nc.sync.dma_start(w2_g, w2r[:, g * 8:(g + 1) * 8, :])
            pe_absorb(w2_g[:, 0, :])
            w2_gs.append(w2_g)

        def out_post(c, sl):
            nc.sync.dma_start(outr[:, c, sl], y1[:, c, sl])

        for j in range(SQ // 512):
            sl = slice(j * 512, (j + 1) * 512)
            for m in range(DC):
                ps = ppE.tile([P, 512], f32, tag="psE", name="psE")
                first = True
                for g in range(FG):
                    for kc in range(DC):
                        nc.tensor.matmul(
                            ps, lhsT=w2_gs[g][:, kc, m * P:(m + 1) * P],
                            rhs=hT[:, g * 8 + kc, sl],
                            start=first, stop=(g == FG - 1 and kc == DC - 1))
                        first = False
                nc.vector.tensor_add(y1[:, m, sl], ps, y1[:, m, sl])
            layernorm_j(y1, y1, g2T, be2T, j, post=out_post)

        ppE_cm.__exit__(None, None, None)
        pffn_cm.__exit__(None, None, None)
        pwE_cm.__exit__(None, None, None)
        px2b_cm.__exit__(None, None, None)
        py1_cm.__exit__(None, None, None)
        constp_cm.__exit__(None, None, None)

    nc.compile()
    return nc


def get_nc():
    if "nc" not in _CACHE:
        _CACHE["nc"] = build_nc()
    return _CACHE["nc"]


def make_in_maps(inputs):
    x = np.ascontiguousarray(np.asarray(inputs["x"], dtype=np.float32))
    shared = {}
    for wname in ("Wq", "Wk", "Wv", "Wo"):
        shared[wname.lower()] = np.ascontiguousarray(
            np.asarray(inputs[wname], dtype=np.float32)).astype(F8)
    for wname in ("W1", "W2"):
        shared[wname.lower()] = np.ascontiguousarray(
            np.asarray(inputs[wname], dtype=np.float32)).astype(BF16)
    cvec = np.zeros((P, 97), dtype=np.float32)
    for i, bname in enumerate(("bq", "bk", "bo", "b2", "g1", "be1",
                               "g2", "be2")):
        arr = np.asarray(inputs[bname], dtype=np.float32)
        cvec[:, i * 8:(i + 1) * 8] = arr.reshape(DC, P).T
    cvec[:, 64:96] = np.asarray(inputs["b1"], np.float32).reshape(FC, P).T
    cvec[:, 96] = EPS
    shared["cvec"] = cvec
    bv = np.asarray(inputs["bv"], dtype=np.float32)
    shared["bvb"] = np.ascontiguousarray(np.broadcast_to(bv, (P, D)))

    in_maps = []
    for core in range(N_CORES):
        b, half = core // 2, core % 2
        xq = x[b, half * SQ:(half + 1) * SQ]        # [SQ, D]
        xqT = np.ascontiguousarray(xq.T)            # [D, SQ]
        xkvT = np.ascontiguousarray(x[b].T)         # [D, SKV]
        m = dict(shared)
        m["xqT"] = xqT
        m["xq8"] = xqT.astype(F8)
        m["xkv8"] = xkvT.astype(F8)
        in_maps.append(m)
    return in_maps


class _Runner:
    """Persistent shard_map runner over the 8 axon cores.

    Mirrors bass2jax.run_bass_via_pjrt but keeps a stable jitted callable so
    repeated kernel() calls don't re-trace, and exposes a timing entry point
    with device-resident inputs.
    """

    def __init__(self, nc):
        import jax
        from jax.sharding import Mesh, PartitionSpec, NamedSharding
        from jax.experimental.shard_map import shard_map
        import concourse.mybir as mybir
        from concourse import bass2jax

        bass2jax.install_neuronx_cc_hook()
        assert nc.dbg_addr is None
        partition_name = (nc.partition_id_tensor.name
                          if nc.partition_id_tensor else None)

        in_names, out_names, out_avals, zero_outs = [], [], [], []
        for alloc in nc.m.functions[0].allocations:
            if not isinstance(alloc, mybir.MemoryLocationSet):
                continue
            name = alloc.memorylocations[0].name
            if alloc.kind == "ExternalInput":
                if name != partition_name:
                    in_names.append(name)
            elif alloc.kind == "ExternalOutput":
                out_names.append(name)
                shape = tuple(alloc.tensor_shape)
                dtype = mybir.dt.np(alloc.dtype)
                out_avals.append(jax.core.ShapedArray(shape, dtype))
                zero_outs.append(np.zeros((N_CORES * shape[0], *shape[1:]),
                                          dtype))
        self.n_params = len(in_names)
        n_outs = len(out_avals)
        all_in_names = in_names + out_names
        if partition_name is not None:
            all_in_names = all_in_names + [partition_name]
        donate = tuple(range(self.n_params, self.n_params + n_outs))

        def _body(*args):
            operands = list(args)
            if partition_name is not None:
                operands.append(bass2jax.partition_id_tensor())
            outs = bass2jax._bass_exec_p.bind(
                *operands,
                out_avals=tuple(out_avals),
                in_names=tuple(all_in_names),
                out_names=tuple(out_names),
                lowering_input_output_aliases=(),
                sim_require_finite=True,
                sim_require_nnan=True,
                nc=nc,
            )
            return tuple(outs)

        devices = jax.devices()[:N_CORES]
        self.mesh = Mesh(np.asarray(devices), ("core",))
        in_specs = (PartitionSpec("core"),) * (self.n_params + n_outs)
        out_specs = (PartitionSpec("core"),) * n_outs
        self.fn = jax.jit(
            shard_map(_body, mesh=self.mesh, in_specs=in_specs,
                      out_specs=out_specs, check_rep=False),
            donate_argnums=donate, keep_unused=True)
        self.sharding = NamedSharding(self.mesh, PartitionSpec("core"))
        self.in_names = in_names
        self.out_names = out_names
        self.out_avals = out_avals
        self.zero_outs = zero_outs
        self.jax = jax

    def concat_inputs(self, in_maps):
        return [np.concatenate([np.asarray(m[name]) for m in in_maps], axis=0)
                for name in self.in_names]

    def put(self, arrs):
        return [self.jax.device_put(a, self.sharding) for a in arrs]

    def run(self, in_maps):
        concat_in = self.concat_inputs(in_maps)
        zeros = self.put(self.zero_outs)
        out_arrs = self.fn(*concat_in, *zeros)
        results = []
        for c in range(N_CORES):
            results.append({
                name: np.asarray(out_arrs[i]).reshape(
                    N_CORES, *self.out_avals[i].shape)[c]
                for i, name in enumerate(self.out_names)})
        return results

    def time_exec(self, in_maps, iters=5):
        """Best-effort device execution time: device-resident inputs,
        pre-staged (donated) zero output buffers, block_until_ready."""
        import time
        concat_in = self.put(self.concat_inputs(in_maps))
        zero_sets = [self.put(self.zero_outs) for _ in range(iters + 1)]
        out = self.fn(*concat_in, *zero_sets[0])  # warm
        self.jax.block_until_ready(out)
        times = []
        for i in range(iters):
            t0 = time.perf_counter()
            out = self.fn(*concat_in, *zero_sets[i + 1])
            self.jax.block_until_ready(out)
            times.append(time.perf_counter() - t0)
        return min(times), times, out


def get_runner():
    if "runner" not in _CACHE:
        _CACHE["runner"] = _Runner(get_nc())
    return _CACHE["runner"]


def run_spmd(inputs, trace=False):
    runner = get_runner()
    in_maps = make_in_maps(inputs)
    results = runner.run(in_maps)
    out = np.empty((B, S, D), dtype=np.float32)
    for core in range(N_CORES):
        b, half = core // 2, core % 2
        out[b, half * SQ:(half + 1) * SQ, :] = results[core]["out"].T
    return out, results


def kernel(**inputs):
    out, _ = run_spmd(inputs)
    return out


if __name__ == "__main__":
    # smoke build
    nc = build_nc()
    print("built ok")
